# revision 1
# baseline (speedup 1.0000x reference)
"""Trainium2 Bass kernel for nn_Mixture_24541443129646.

loss(x, mu, prec) = -sum_n logsumexp_k( -0.5 * sum_d prec_d (x[n,d]-mu[k,d])^2 )

Math used here (exact algebra):
  m[n,k]  = cross[n,k] - 0.5*x_sq[n] - 0.5*mu_sq[k]
  lse[n]  = -0.5*x_sq[n] + log( sum_k exp(cross[n,k] - 0.5*mu_sq[k]) )
  loss    = 0.5*S_xx - sum_n log(rowsum[n]),   S_xx = sum_n x_sq[n]

Per-core device work (data-parallel over N, 8 cores):
  - cross via PE matmuls (float32r, 1 cyc/row):  psum[128 rows, 256 k]
  - mu_sq fold via rank-1 matmul accumulate (ones^T (-0.5*mu_sq))
  - exp on ScalarE (PSUM -> SBUF bf16)
  - per-row sums over k: DVE bf16 pairwise tree + fp32 tail reduce
  - S_xx partials via DVE tensor_tensor_reduce on the raw x^T shard
  - log on ScalarE, final row reduction on DVE
Host: shard/transpose x, fold prec into mu, sum the 8 per-core partials.
"""

import sys

sys.path.insert(0, "/opt/trn_rl_repo")

from contextlib import ExitStack

import numpy as np

import concourse.bass as bass
import concourse.tile as tile
from concourse import mybir
from concourse.bass_utils import run_bass_kernel_spmd


N, K, D = 131072, 256, 128
NCORES = 8
RPC = N // NCORES  # rows per core = 16384
CHUNK = 2048  # x^T DMA chunk (columns = rows of x)
NCHUNK = RPC // CHUNK  # 8
NCHUNK_DVE = 2  # x^2 chunks reduced on DVE; the rest go to ScalarE
TILE_R = 128  # row tile (psum partition dim)
BATCH_ROWS = 1024  # rows per psum batch (8 row tiles, 4 psum banks)
TILES_PER_BATCH = BATCH_ROWS // TILE_R  # 8
NBATCH = RPC // BATCH_ROWS  # 16
ROUND_BATCHES = 4  # psum batches per exp/reduce round
NROUND = NBATCH // ROUND_BATCHES  # 4
SEGS = ROUND_BATCHES * TILES_PER_BATCH  # 32 k-segments per round
EPL = ROUND_BATCHES * BATCH_ROWS * K // 128  # 8192 e-elems per lane per round

CSHIFT = 40.0  # exp shift: w = exp(C - musq/2); lse = log(rowsum) - C

F32 = mybir.dt.float32
F32R = mybir.dt.float32r
BF16 = mybir.dt.bfloat16
ALU = mybir.AluOpType
ACTF = mybir.ActivationFunctionType


def _split_excess_waits(nc, max_waits=1):
    """This container's walrus rejects >max_waits sem-waits on one
    instruction ("Too many sync wait commands", CoreV3GenImpl setupSyncWait).
    Move the excess onto helper Drain instructions inserted just before, on
    the same engine — semantically identical (all waits still complete before
    the original instruction executes)."""
    import bass_rust

    n_fix = 0
    for f in nc.m.functions:
        for bb in f.blocks:
            insts = bb.instructions
            out_list = []
            changed = False
            for ins in insts:
                si = ins.sync_info
                if si is not None and len(si.on_wait) > max_waits:
                    waits = list(si.on_wait)
                    extra, keep = waits[:-max_waits], waits[-max_waits:]
                    for i in range(0, len(extra), max_waits):
                        nd = mybir.InstDrain(name=f"I-waitfix-{n_fix}", ins=[], outs=[])
                        n_fix += 1
                        nd.engine = ins.engine
                        nd.sync_info = bass_rust.SyncInfo(
                            on_wait=extra[i : i + max_waits], on_update=[]
                        )
                        out_list.append(nd)
                    si.on_wait = keep
                    changed = True
                out_list.append(ins)
            if changed:
                bb.instructions = out_list
    return n_fix


def build_program(apply_waitfix=True):
    nc = bass.Bass("TRN2", target_bir_lowering=False, debug=False)

    xt = nc.dram_tensor("xt", [D, RPC], BF16, kind="ExternalInput").ap()
    mupt = nc.dram_tensor("mupt", [D, K], BF16, kind="ExternalInput").ap()
    wbc = nc.dram_tensor("wbc", [128, BATCH_ROWS * K // 128], BF16, kind="ExternalInput").ap()
    out = nc.dram_tensor("out", [128, 3], F32, kind="ExternalOutput").ap()

    with tile.TileContext(nc) as tc:
        with ExitStack() as ctx:
            const_pool = ctx.enter_context(tc.tile_pool(name="const", bufs=1))
            xt_pool = ctx.enter_context(tc.tile_pool(name="xtp", bufs=1))
            ps_pool = ctx.enter_context(tc.tile_pool(name="ps", bufs=2, space="PSUM"))
            e_pool = ctx.enter_context(tc.tile_pool(name="e", bufs=2))
            tree_pool = ctx.enter_context(tc.tile_pool(name="tree", bufs=2))
            sq_pool = ctx.enter_context(tc.tile_pool(name="sq", bufs=2))
            misc_pool = ctx.enter_context(tc.tile_pool(name="misc", bufs=1))

            mupt_sb = const_pool.tile([D, K], BF16, tag="mupt")
            nc.sync.dma_start(mupt_sb[:], mupt)
            wbc_sb = const_pool.tile([128, BATCH_ROWS * K // 128], BF16, tag="wbc")
            nc.sync.dma_start(wbc_sb[:], wbc)

            out_sb = misc_pool.tile([128, 3], F32, tag="out")
            rowsum = misc_pool.tile([128, NBATCH * TILES_PER_BATCH], F32, tag="rsum")
            loglse = misc_pool.tile([128, NBATCH * TILES_PER_BATCH], F32, tag="llse")

            # x^T shard load (8 chunks) + S_xx partials per chunk.
            # x^2 accumulators: DVE handles NCHUNK_DVE chunks (accum cols in
            # xsq_dve), ScalarE Square+accum handles the rest (xsq_act cols);
            # both are summed per-partition into out_sb cols 1..2 at the end.
            xsq_dve = misc_pool.tile([128, NCHUNK_DVE], F32, tag="xsqd")
            xsq_act = misc_pool.tile([128, NCHUNK - NCHUNK_DVE], F32, tag="xsqa")
            xt_sb = []
            for c in range(NCHUNK):
                t = xt_pool.tile([D, CHUNK], BF16, tag=f"xt{c}")
                nc.sync.dma_start(t[:], xt[:, c * CHUNK : (c + 1) * CHUNK])
                xt_sb.append(t)
            for c in range(NCHUNK):
                if c < NCHUNK_DVE:
                    sq = sq_pool.tile([D, CHUNK], BF16, tag="sq")
                    nc.vector.scalar_tensor_tensor(
                        out=sq[:],
                        in0=xt_sb[c][:],
                        scalar=1.0,
                        in1=xt_sb[c][:],
                        op0=ALU.mult,
                        op1=ALU.mult,
                        accum_out=xsq_dve[:, c : c + 1],
                    )
                else:
                    sqa = sq_pool.tile([D, CHUNK], BF16, tag="sqa")
                    nc.scalar.activation(
                        sqa[:],
                        xt_sb[c][:],
                        ACTF.Square,
                        accum_out=xsq_act[:, c - NCHUNK_DVE : c - NCHUNK_DVE + 1],
                    )

            for r in range(NROUND):
                e_rt = e_pool.tile([128, EPL], BF16, tag="e")
                for bb in range(ROUND_BATCHES):
                    b = r * ROUND_BATCHES + bb
                    ps = ps_pool.tile([128, TILES_PER_BATCH * K], F32, tag="ps")
                    for j in range(TILES_PER_BATCH):
                        row0 = b * BATCH_ROWS + j * TILE_R
                        ci, off = divmod(row0, CHUNK)
                        pj = ps[:, j * K : (j + 1) * K]
                        nc.tensor.matmul(
                            pj,
                            lhsT=xt_sb[ci][:, off : off + TILE_R],
                            rhs=mupt_sb[:],
                            start=True,
                            stop=True,
                        )
                    nc.scalar.activation(
                        e_rt[:, bb * (BATCH_ROWS * K // 128) : (bb + 1) * (BATCH_ROWS * K // 128)],
                        ps[:],
                        ACTF.Exp,
                    )

                # weight by w[k] = exp(C - musq[k]/2), then pairwise bf16 tree
                # over each 256-wide k segment: 256 -> 8, fp32 tail.
                wp = tree_pool.tile([128, EPL], BF16, tag="wp")
                BW = BATCH_ROWS * K // 128
                for bb in range(ROUND_BATCHES):
                    nc.vector.tensor_mul(
                        wp[:, bb * BW : (bb + 1) * BW],
                        e_rt[:, bb * BW : (bb + 1) * BW],
                        wbc_sb[:],
                    )
                cur = wp[:].rearrange("p (s k) -> p s k", k=K)
                w = K // 2
                li = 0
                while w >= 8:
                    tl = tree_pool.tile([128, SEGS * w], BF16, tag=f"t{li}")
                    dst = tl[:].rearrange("p (s k) -> p s k", k=w)
                    nc.vector.tensor_add(dst, cur[:, :, 0:w], cur[:, :, w : 2 * w])
                    cur = dst
                    w //= 2
                    li += 1
                nc.vector.tensor_reduce(
                    rowsum[:, r * SEGS : (r + 1) * SEGS],
                    cur,
                    axis=mybir.AxisListType.X,
                    op=ALU.add,
                )

            nc.scalar.activation(loglse[:], rowsum[:], ACTF.Ln)
            nc.vector.tensor_reduce(
                out_sb[:, 0:1], loglse[:], axis=mybir.AxisListType.X, op=ALU.add
            )
            nc.vector.tensor_reduce(
                out_sb[:, 1:2], xsq_dve[:], axis=mybir.AxisListType.X, op=ALU.add
            )
            nc.vector.tensor_reduce(
                out_sb[:, 2:3], xsq_act[:], axis=mybir.AxisListType.X, op=ALU.add
            )
            nc.sync.dma_start(out, out_sb[:])

    if apply_waitfix:
        _split_excess_waits(nc)
    return nc


def make_in_maps(x, mu, prec):
    import ml_dtypes

    x = np.asarray(x, dtype=np.float32)
    mu = np.asarray(mu, dtype=np.float32)
    prec = np.asarray(prec, dtype=np.float32)
    mupt = np.ascontiguousarray((mu * prec[None, :]).T).astype(ml_dtypes.bfloat16)
    musq_half = 0.5 * ((mu * mu) @ prec)  # [K]
    w = np.exp(CSHIFT - musq_half.astype(np.float64)).astype(np.float32)
    wbc = np.broadcast_to(
        np.tile(w, BATCH_ROWS * K // 128 // K)[None, :], (128, BATCH_ROWS * K // 128)
    ).astype(ml_dtypes.bfloat16)
    wbc = np.ascontiguousarray(wbc)
    in_maps = []
    for c in range(NCORES):
        xt_c = np.ascontiguousarray(x[c * RPC : (c + 1) * RPC, :].T).astype(
            ml_dtypes.bfloat16
        )  # [D, RPC]
        in_maps.append({"xt": xt_c, "mupt": mupt, "wbc": wbc})
    return in_maps


def combine_outputs(outs, prec):
    """outs: list of 8 [128, 3] arrays. col0 = per-partition sum of
    log(sum_k exp(cross - musq/2 + C)), cols 1..2 = per-d partial sums of
    x^2 (DVE part, ScalarE part)."""
    prec = np.asarray(prec, dtype=np.float64)
    lse_sum = 0.0
    s_xx = 0.0
    for o in outs:
        o = np.asarray(o, dtype=np.float64)
        lse_sum += o[:, 0].sum()
        s_xx += (o[:, 1:3].sum(axis=1) * prec).sum()
    total = 0.5 * s_xx - (lse_sum - N * CSHIFT)
    return np.float32(total)


_CACHED_NC = None


def kernel(x, mu, prec):
    global _CACHED_NC
    if _CACHED_NC is None:
        _CACHED_NC = build_program()
    nc = _CACHED_NC
    in_maps = make_in_maps(x, mu, prec)
    res = run_bass_kernel_spmd(nc, in_maps, core_ids=list(range(NCORES)))
    outs = [res.results[c]["out"] for c in range(NCORES)]
    return combine_outputs(outs, prec)


if __name__ == "__main__":
    import reference

    inputs = {k: np.asarray(v) for k, v in reference.setup_inputs().items()}
    expected = float(reference.reference(**inputs))
    actual = float(kernel(**inputs))
    rel = abs(actual - expected) / max(1.0, abs(expected))
    print(f"expected={expected:.6f} actual={actual:.6f} rel={rel:.3e}")



# revision 2
# speedup vs baseline: 1.0042x; 1.0042x over previous
"""Trainium2 Bass kernel v3 for nn_Mixture_24541443129646.

loss = 0.5*S_xx - sum_n log sum_k exp(cross[n,k] - musq[k]/2)   (+ N*C shift)

Transposed layout (k on partitions), data-parallel over N on 8 cores:
  PE:   cross halves psum_h [128k, 1024n] = mupt_h^T @ xt chunk
  ACT:  e_h = Exp(psum_h + b_h), b_h = C - musq/2 per-partition bias
  DVE:  e01 = e0 + e1; drain rowsums psum -> SBUF
  PE:   rowsums rs [8, 1024] = ones^T @ e01
  DMA:  bounce rowsums through flat DRAM to respread n over partitions
  ACT:  Ln + accumulate
Host: shard/transpose/cast x, fold prec into mu, S_xx, final scalar.
"""

import sys

sys.path.insert(0, "/opt/trn_rl_repo")

from contextlib import ExitStack

import numpy as np

import concourse.bass as bass
import concourse.tile as tile
from concourse import mybir
from concourse.bass_utils import run_bass_kernel_spmd

N, K, D = 131072, 256, 128
NCORES = 8
RPC = N // NCORES
CW = 1024
NCHUNK = RPC // CW
MMW = 512
CSHIFT = 40.0

F32 = mybir.dt.float32
BF16 = mybir.dt.bfloat16
ACTF = mybir.ActivationFunctionType

XT_LOADS = [1024, 1024, 2048, 4096, 4096, 4096]
assert sum(XT_LOADS) == RPC


def _split_excess_waits(nc, max_waits=1):
    import bass_rust

    n_fix = 0
    for f in nc.m.functions:
        for bb in f.blocks:
            insts = bb.instructions
            out_list = []
            changed = False
            for ins in insts:
                si = ins.sync_info
                if si is not None and len(si.on_wait) > max_waits:
                    waits = list(si.on_wait)
                    extra, keep = waits[:-max_waits], waits[-max_waits:]
                    for i in range(0, len(extra), max_waits):
                        nd = mybir.InstDrain(name=f"I-waitfix-{n_fix}", ins=[], outs=[])
                        n_fix += 1
                        nd.engine = ins.engine
                        nd.sync_info = bass_rust.SyncInfo(
                            on_wait=extra[i : i + max_waits], on_update=[]
                        )
                        out_list.append(nd)
                    si.on_wait = keep
                    changed = True
                out_list.append(ins)
            if changed:
                bb.instructions = out_list
    return n_fix


def build_program(apply_waitfix=True):
    nc = bass.Bass("TRN2", target_bir_lowering=False, debug=False)

    xt = nc.dram_tensor("xt", [D, RPC], BF16, kind="ExternalInput").ap()
    mupt = nc.dram_tensor("mupt", [D, K], BF16, kind="ExternalInput").ap()
    bc = nc.dram_tensor("bc", [128, 2], F32, kind="ExternalInput").ap()
    o8 = nc.dram_tensor("o8", [128, 8], BF16, kind="ExternalInput").ap()
    rs_dram = nc.dram_tensor("rs_dram", [NCHUNK, 8, CW], F32, kind="Internal").ap()
    out = nc.dram_tensor("out", [128, 2], F32, kind="ExternalOutput").ap()

    with tile.TileContext(nc) as tc:
        with ExitStack() as ctx:
            cpool = ctx.enter_context(tc.tile_pool(name="const", bufs=1))
            xpool = ctx.enter_context(tc.tile_pool(name="xt", bufs=1))
            pA = ctx.enter_context(tc.tile_pool(name="pA", bufs=1, space="PSUM"))
            pB = ctx.enter_context(tc.tile_pool(name="pB", bufs=1, space="PSUM"))
            pR = ctx.enter_context(tc.tile_pool(name="pR", bufs=2, space="PSUM"))
            epool = ctx.enter_context(tc.tile_pool(name="e", bufs=4))
            e01pool = ctx.enter_context(tc.tile_pool(name="e01", bufs=3))
            rspool = ctx.enter_context(tc.tile_pool(name="rs", bufs=3))
            mpool = ctx.enter_context(tc.tile_pool(name="misc", bufs=1))

            # first xt chunk first: it gates the first exp
            xt_sb = []
            col = 0
            t0 = xpool.tile([D, XT_LOADS[0]], BF16, tag="xt0")
            nc.sync.dma_start(t0[:], xt[:, 0 : XT_LOADS[0]])
            xt_sb.append((t0, 0, XT_LOADS[0]))
            col = XT_LOADS[0]

            mupt_sb = cpool.tile([D, K], BF16, tag="mupt")
            nc.sync.dma_start(mupt_sb[:], mupt)
            bc_sb = cpool.tile([128, 2], F32, tag="bc")
            nc.sync.dma_start(bc_sb[:], bc)

            for li, w in enumerate(XT_LOADS[1:], start=1):
                t = xpool.tile([D, w], BF16, tag=f"xt{li}")
                nc.sync.dma_start(t[:], xt[:, col : col + w])
                xt_sb.append((t, col, w))
                col += w

            o8_sb = cpool.tile([128, 8], BF16, tag="o8")
            nc.sync.dma_start(o8_sb[:], o8)

            def xt_slice(c0, w):
                for t, s, n in xt_sb:
                    if s <= c0 and c0 + w <= s + n:
                        return t[:, c0 - s : c0 - s + w]
                raise AssertionError("chunk crosses load boundary")

            lnv = mpool.tile([128, NCHUNK * 8], F32, tag="lnv")
            out_sb = mpool.tile([128, 2], F32, tag="out")
            rsd = mpool.tile([128, NCHUNK * 8], F32, tag="rsd")

            # prewarm the Exp ACT table so the first exp skips the 1.3us load
            warm = mpool.tile([128, 1], F32, tag="warm")
            nc.vector.memset(warm[:], 0.0)
            nc.scalar.activation(warm[:], warm[:], ACTF.Exp)

            prev = None
            prev_reload = None
            for j in range(NCHUNK):
                psA = pA.tile([128, CW], F32, tag="A")
                psB = pB.tile([128, CW], F32, tag="B")
                for o in range(0, CW, MMW):
                    nc.tensor.matmul(
                        psA[:, o : o + MMW],
                        lhsT=mupt_sb[:, 0:128],
                        rhs=xt_slice(j * CW + o, MMW),
                        start=True,
                        stop=True,
                    )
                for o in range(0, CW, MMW):
                    nc.tensor.matmul(
                        psB[:, o : o + MMW],
                        lhsT=mupt_sb[:, 128:256],
                        rhs=xt_slice(j * CW + o, MMW),
                        start=True,
                        stop=True,
                    )
                e0 = epool.tile([128, CW], BF16, tag="e0")
                e1 = epool.tile([128, CW], BF16, tag="e1")
                nc.scalar.activation(e0[:], psA[:], ACTF.Exp, bias=bc_sb[:, 0:1])
                nc.scalar.activation(e1[:], psB[:], ACTF.Exp, bias=bc_sb[:, 1:2])
                e01 = e01pool.tile([128, CW], BF16, tag="e01")
                nc.vector.tensor_add(e01[:], e0[:], e1[:])

                if prev is not None:
                    _ones_drain_bounce(nc, o8_sb, rspool, pR, rs_dram, prev)
                if prev_reload is not None:
                    _reload(nc, rs_dram, rsd, prev_reload)
                prev_reload = prev[1] if prev is not None else None
                prev = (e01, j)

            _ones_drain_bounce(nc, o8_sb, rspool, pR, rs_dram, prev)
            if prev_reload is not None:
                _reload(nc, rs_dram, rsd, prev_reload)
            _reload(nc, rs_dram, rsd, prev[1])

            nc.scalar.activation(lnv[:], rsd[:], ACTF.Ln, accum_out=out_sb[:, 0:1])
            nc.vector.memset(out_sb[:, 1:2], 0.0)
            nc.sync.dma_start(out, out_sb[:])

    if apply_waitfix:
        _split_excess_waits(nc)
    return nc


def _ones_drain_bounce(nc, o8_sb, rspool, pR, rs_dram, prev):
    e01, j = prev
    rs = pR.tile([8, CW], F32, tag="rs")
    for o in range(0, CW, MMW):
        nc.tensor.matmul(
            rs[:, o : o + MMW],
            lhsT=o8_sb[:],
            rhs=e01[:, o : o + MMW],
            start=True,
            stop=True,
        )
    rssb = rspool.tile([8, CW], F32, tag="rssb")
    nc.vector.tensor_copy(rssb[:], rs[:])
    nc.sync.dma_start(rs_dram[j], rssb[:])


def _reload(nc, rs_dram, rsd, j):
    src = rs_dram[j, 0:1, :].rearrange("o (p i) -> (o p) i", p=128)
    nc.sync.dma_start(rsd[:, j * 8 : (j + 1) * 8], src)


def make_in_maps(x, mu, prec):
    import ml_dtypes

    x = np.asarray(x, dtype=np.float32)
    mu = np.asarray(mu, dtype=np.float32)
    prec = np.asarray(prec, dtype=np.float32)
    mupt = np.ascontiguousarray((mu * prec[None, :]).T).astype(ml_dtypes.bfloat16)
    musq_half = 0.5 * ((mu.astype(np.float64) ** 2) @ prec.astype(np.float64))
    bc = np.empty((128, 2), np.float32)
    bc[:, 0] = (CSHIFT - musq_half[0:128]).astype(np.float32)
    bc[:, 1] = (CSHIFT - musq_half[128:256]).astype(np.float32)
    o8 = np.ones((128, 8), np.float32).astype(ml_dtypes.bfloat16)
    in_maps = []
    for c in range(NCORES):
        xt_c = np.ascontiguousarray(x[c * RPC : (c + 1) * RPC, :].T).astype(
            ml_dtypes.bfloat16
        )
        in_maps.append({"xt": xt_c, "mupt": mupt, "bc": bc, "o8": o8})
    return in_maps


def combine_outputs(outs, x, prec):
    x64 = np.asarray(x, dtype=np.float64)
    prec64 = np.asarray(prec, dtype=np.float64)
    s_xx = float(((x64 * x64) @ prec64).sum())
    lse_sum = 0.0
    for o in outs:
        lse_sum += np.asarray(o, dtype=np.float64)[:, 0:2].sum()
    total = 0.5 * s_xx - (lse_sum - N * CSHIFT)
    return np.float32(total)


_CACHED_NC = None


def kernel(x, mu, prec):
    global _CACHED_NC
    if _CACHED_NC is None:
        _CACHED_NC = build_program()
    nc = _CACHED_NC
    in_maps = make_in_maps(x, mu, prec)
    res = run_bass_kernel_spmd(nc, in_maps, core_ids=list(range(NCORES)))
    outs = [res.results[c]["out"] for c in range(NCORES)]
    return combine_outputs(outs, x, prec)


if __name__ == "__main__":
    import reference

    inputs = {k: np.asarray(v) for k, v in reference.setup_inputs().items()}
    expected = float(reference.reference(**inputs))
    actual = float(kernel(**inputs))
    rel = abs(actual - expected) / max(1.0, abs(expected))
    print(f"expected={expected:.6f} actual={actual:.6f} rel={rel:.3e}")
